# revision 22
# baseline (speedup 1.0000x reference)
"""Trainium2 Bass kernel for batched dense attention.

Problem shapes (hardcoded): B=16, Lq=Lk=2048, E=1024, f32 I/O.
Sharding: batch dim across 8 NeuronCores (2 batches per core), no
communication. Each core computes, per batch:
    Q = ctx @ Wq^T ; K = x @ Wk^T ; V = x @ Wv^T
    att = softmax(Q K^T / 32) ; out = (att @ V) @ Wo^T + bo
Returns (out, att) like the reference.

Compute is bf16 on TensorE (fp32 PSUM accumulation). Input-side
transposes (weights/x/ctx, needed to put the contraction dim on
partitions) run on the TensorEngine as f32 PE-transposes, inline with
the matmul stream; P transposes ride the DMA X-bar from the Sync FIFO
(the only X-bar user in the attention phase). Softmax skips the
max-subtraction (scores are ~N(0,1); exp cannot overflow) and defers
the 1/rowsum normalisation of the out-path to the final projection
epilogue, where it is a per-partition scalar. Attention supers are
software-pipelined: super s's O/out-proj matmuls are emitted after
super s+1's S matmuls.
"""

import numpy as np

import concourse.bass as bass
import concourse.mybir as mybir
import concourse.tile as tile
from concourse import bacc
from concourse.bass_utils import run_bass_kernel_spmd
from concourse.masks import make_identity

F32 = mybir.dt.float32
BF16 = mybir.dt.bfloat16

B = 16
CORES = 8
BPC = B // CORES          # batches per core
LQ = 2048
LK = 2048
E = 1024
P = 128
NE = E // P               # 8  e-chunks
NF = E // P               # 8  f-chunks
NKC = LK // P             # 16 k-chunks
NSL = LK // 512           # 4  k-slices of 512
NSUP = LQ // 512          # 4  q-supers of 512
SCALE = 1.0 / 32.0        # 1/sqrt(E)


def build():
    nc = bacc.Bacc("TRN2", target_bir_lowering=False, debug=False)

    x = nc.dram_tensor("x", [BPC, LK, E], F32, kind="ExternalInput").ap()
    ctx = nc.dram_tensor("context", [BPC, LQ, E], F32, kind="ExternalInput").ap()
    Wq = nc.dram_tensor("Wq", [E, E], F32, kind="ExternalInput").ap()
    Wk = nc.dram_tensor("Wk", [E, E], F32, kind="ExternalInput").ap()
    Wv = nc.dram_tensor("Wv", [E, E], F32, kind="ExternalInput").ap()
    Wo = nc.dram_tensor("Wo", [E, E], F32, kind="ExternalInput").ap()
    bo = nc.dram_tensor("bo", [E], F32, kind="ExternalInput").ap()
    out = nc.dram_tensor("out", [BPC, LQ, E], F32, kind="ExternalOutput").ap()
    att = nc.dram_tensor("att", [BPC, LQ, LK], F32, kind="ExternalOutput").ap()

    # DRAM scratch holding bf16 weight rows; batch 1 X-bar-transposes them
    # straight from DRAM on the Sync FIFO.
    wbf_scr = nc.dram_tensor("wbf_scr", [4, NF, P, E], BF16).ap()

    with tile.TileContext(nc) as tc:
        with tc.tile_pool(name="const", bufs=1) as const:
            bo_bc = const.tile([P, E], F32, tag="bo")
            bo_b = bass.AP(tensor=bo.tensor, offset=bo.offset,
                           ap=[[0, P]] + list(bo.ap))
            nc.gpsimd.dma_start(out=bo_bc, in_=bo_b)
            idf = const.tile([P, P], F32, tag="idf")
            make_identity(nc, idf)

            with tc.tile_pool(name="qkv", bufs=1) as qkv:
                for b in range(BPC):
                    QT = qkv.tile([P, NF, LQ], BF16, tag="QT")
                    KT = qkv.tile([P, NF, LK], BF16, tag="KT")
                    Vt = qkv.tile([P, NKC, E], BF16, tag="V")

                    # WoT outlives the projection phase (used in attention).
                    with tc.tile_pool(name=f"wo{b}", bufs=1) as wop:
                        WoT = wop.tile([P, NE, E], BF16, tag="WoT")
                        attention_batch(nc, tc, b, x, ctx, Wq, Wk, Wv, Wo,
                                        wbf_scr, out, att, bo_bc, idf,
                                        QT, KT, Vt, WoT)

    nc.compile()
    return nc


def attention_batch(nc, tc, b, x, ctx, Wq, Wk, Wv, Wo, wbf_scr,
                    out, att, bo_bc, idf, QT, KT, Vt, WoT):
    # ================= projection phase =================
    with tc.tile_pool(name=f"stg{b}", bufs=4) as stg, \
         tc.tile_pool(name=f"wt{b}", bufs=1) as wt, \
         tc.tile_pool(name=f"xt{b}", bufs=2) as xtp, \
         tc.tile_pool(name=f"pa{b}", bufs=6, space="PSUM") as pacc, \
         tc.tile_pool(name=f"pt{b}", bufs=2, space="PSUM") as ptr:

        WkT = wt.tile([P, NE, E], BF16, tag="WkT")
        WvT = wt.tile([P, NE, E], BF16, tag="WvT")
        WqT = wt.tile([P, NE, E], BF16, tag="WqT")

        copy_flip = [0]

        def pe_transpose(dst, src_f32, ec):
            """PE f32 transpose of one [P, P] block into bf16 dst, with the
            PSUM->SBUF copy alternating between DVE and ACT."""
            pt = ptr.tile([P, P], F32, tag="ptr")
            nc.tensor.transpose(pt, src_f32[:, ec * P:(ec + 1) * P], idf)
            eng = nc.vector if copy_flip[0] % 2 == 0 else nc.scalar
            copy_flip[0] += 1
            if eng is nc.vector:
                nc.vector.tensor_copy(dst, pt)
            else:
                nc.scalar.copy(dst, pt)

        def w_blocks(wi, wap, wT):
            """Stage f32 rows of a weight, PE-transpose into bf16 wT, and
            spill bf16 rows to scratch for batch 1."""
            for fb in range(NF):
                wrow = stg.tile([P, E], F32, tag="xs")
                nc.gpsimd.dma_start(out=wrow[:64], in_=wap[fb * P:fb * P + 64, :])
                nc.gpsimd.dma_start(out=wrow[64:], in_=wap[fb * P + 64:(fb + 1) * P, :])
                for ec in range(NE):
                    pe_transpose(wT[:, ec, fb * P:(fb + 1) * P], wrow, ec)
                wrbf = stg.tile([P, E], BF16, tag="xb")
                nc.vector.tensor_copy(wrbf, wrow)
                nc.gpsimd.dma_start(out=wbf_scr[wi, fb], in_=wrbf)

        def w_load(wi, wT, fbs):
            """Batch 1: X-bar transpose bf16 weight rows straight from DRAM
            scratch on the Sync FIFO — no SWDGE or DVE involvement."""
            for fb in fbs:
                nc.sync.dma_start_transpose(
                    wT[:, :, fb * P:(fb + 1) * P], wbf_scr[wi, fb])

        def stage_T(src_dram, row0, xT, kb):
            """DRAM f32 rows -> bf16 -> X-bar transpose into xT[:, :, kb]."""
            xs = stg.tile([P, E], F32, tag="xs")
            nc.gpsimd.dma_start(out=xs[:64], in_=src_dram[b, row0:row0 + 64, :])
            nc.gpsimd.dma_start(out=xs[64:],
                                in_=src_dram[b, row0 + 64:row0 + P, :])
            xb = stg.tile([P, E], BF16, tag="xb")
            nc.vector.tensor_copy(xb, xs)
            nc.sync.dma_start_transpose(xT[:, :, kb * P:(kb + 1) * P], xb)

        def x_slice(sl):
            xT = xtp.tile([P, NE, 512], BF16, tag="xT")
            for kb in range(4):
                stage_T(x, sl * 512 + kb * P, xT, kb)
            for fb in range(NF):
                acc = pacc.tile([P, 512], F32, tag="acc")
                for ec in range(NE):
                    nc.tensor.matmul(
                        acc, WkT[:, ec, fb * P:(fb + 1) * P], xT[:, ec, :],
                        start=(ec == 0), stop=(ec == NE - 1))
                nc.scalar.copy(KT[:, fb, sl * 512:(sl + 1) * 512], acc)
            for kb in range(4):
                for fh in range(2):
                    acc = pacc.tile([P, 512], F32, tag="acc")
                    for ec in range(NE):
                        nc.tensor.matmul(
                            acc, xT[:, ec, kb * P:(kb + 1) * P],
                            WvT[:, ec, fh * 512:(fh + 1) * 512],
                            start=(ec == 0), stop=(ec == NE - 1))
                    nc.scalar.copy(
                        Vt[:, sl * 4 + kb, fh * 512:(fh + 1) * 512], acc)

        def ctx_slice(sl):
            cT = xtp.tile([P, NE, 512], BF16, tag="xT")
            for qb in range(4):
                stage_T(ctx, sl * 512 + qb * P, cT, qb)
            for fb in range(NF):
                acc = pacc.tile([P, 512], F32, tag="acc")
                for ec in range(NE):
                    nc.tensor.matmul(
                        acc, WqT[:, ec, fb * P:(fb + 1) * P], cT[:, ec, :],
                        start=(ec == 0), stop=(ec == NE - 1))
                nc.scalar.copy(QT[:, fb, sl * 512:(sl + 1) * 512], acc)

        if b == 0:
            w_blocks(1, Wk, WkT)
            w_blocks(2, Wv, WvT)
        else:
            w_load(1, WkT, range(NF))
            w_load(2, WvT, range(NF))
        x_slice(0)
        x_slice(1)
        if b == 0:
            w_blocks(0, Wq, WqT)
        else:
            w_load(0, WqT, range(NF))
        ctx_slice(0)
        x_slice(2)
        ctx_slice(1)
        x_slice(3)
        if b == 0:
            w_blocks(3, Wo, WoT)
        else:
            w_load(3, WoT, range(NF))
        ctx_slice(2)
        ctx_slice(3)

    # ================= attention phase =================
    with tc.tile_pool(name=f"pp{b}", bufs=2) as pp, \
         tc.tile_pool(name=f"at{b}", bufs=2) as atp, \
         tc.tile_pool(name=f"ot{b}", bufs=2) as otp, \
         tc.tile_pool(name=f"af{b}", bufs=6) as afp, \
         tc.tile_pool(name=f"sm{b}", bufs=12) as small, \
         tc.tile_pool(name=f"pS{b}", bufs=3, space="PSUM") as psS, \
         tc.tile_pool(name=f"pO{b}", bufs=3, space="PSUM") as psO, \
         tc.tile_pool(name=f"pF{b}", bufs=2, space="PSUM") as psF:

        def softmax_super(sup):
            """S matmuls + exp + att store + P transposes for one q-super.
            Returns (attT, recs) consumed by o_proj_super."""
            attT = atp.tile([P, NKC, 512], BF16, tag="attT")
            recs = []
            for qq in range(4):
                qb = sup * 4 + qq
                Pt = pp.tile([P, LK], BF16, tag="P")
                prt = small.tile([P, NSL], F32, tag="prt")
                for sl in range(NSL):
                    S = psS.tile([P, 512], F32, tag="S")
                    for fc in range(NF):
                        nc.tensor.matmul(
                            S, QT[:, fc, qb * P:(qb + 1) * P],
                            KT[:, fc, sl * 512:(sl + 1) * 512],
                            start=(fc == 0), stop=(fc == NF - 1))
                    nc.scalar.activation(
                        out=Pt[:, sl * 512:(sl + 1) * 512], in_=S,
                        func=mybir.ActivationFunctionType.Exp,
                        scale=SCALE, accum_out=prt[:, sl:sl + 1])
                    if sl % 2 == 1:
                        # transpose the finished [P, 1024] half; the Sync
                        # FIFO carries only transposes in this phase
                        half = sl // 2
                        nc.sync.dma_start_transpose(
                            attT[:, half * NE:(half + 1) * NE,
                                 qq * P:(qq + 1) * P],
                            Pt[:, half * 1024:(half + 1) * 1024])
                rsum = small.tile([P, 1], F32, tag="rsum")
                nc.vector.reduce_sum(rsum, prt, axis=mybir.AxisListType.X)
                rec = small.tile([P, 1], F32, tag="rec")
                nc.vector.reciprocal(rec, rsum)
                recs.append(rec)
                for sl in range(NSL):
                    af = afp.tile([P, 512], F32, tag="af")
                    nc.vector.tensor_scalar_mul(
                        af, Pt[:, sl * 512:(sl + 1) * 512], rec)
                    nc.gpsimd.dma_start(
                        out=att[b, qb * P:(qb + 1) * P,
                                sl * 512:(sl + 1) * 512],
                        in_=af)
            return attT, recs

        def o_proj_super(sup, attT, recs):
            """attV matmuls + output projection for one q-super."""
            OT = otp.tile([P, NE, 512], BF16, tag="OT")
            for eb in range(NE):
                acc = psO.tile([P, 512], F32, tag="O")
                for kc in range(NKC):
                    nc.tensor.matmul(
                        acc, Vt[:, kc, eb * P:(eb + 1) * P], attT[:, kc, :],
                        start=(kc == 0), stop=(kc == NKC - 1))
                nc.vector.tensor_copy(OT[:, eb, :], acc)
            for qq in range(4):
                qb = sup * 4 + qq
                for fh in range(2):
                    acc = psF.tile([P, 512], F32, tag="F")
                    for ec in range(NE):
                        nc.tensor.matmul(
                            acc, OT[:, ec, qq * P:(qq + 1) * P],
                            WoT[:, ec, fh * 512:(fh + 1) * 512],
                            start=(ec == 0), stop=(ec == NE - 1))
                    ost = afp.tile([P, 512], F32, tag="af")
                    nc.vector.tensor_scalar_mul(ost, acc, recs[qq])
                    nc.vector.tensor_add(
                        ost, ost, bo_bc[:, fh * 512:(fh + 1) * 512])
                    nc.gpsimd.dma_start(
                        out=out[b, qb * P:(qb + 1) * P,
                                fh * 512:(fh + 1) * 512],
                        in_=ost)

        pending = None
        for sup in range(NSUP):
            cur = softmax_super(sup)
            if pending is not None:
                o_proj_super(sup - 1, *pending)
            pending = cur
        o_proj_super(NSUP - 1, *pending)


_CACHE = {}


def _get_nc():
    if "nc" not in _CACHE:
        _CACHE["nc"] = build()
    return _CACHE["nc"]


def kernel(x, context, Wq, Wk, Wv, Wo, bo):
    x = np.ascontiguousarray(np.asarray(x, dtype=np.float32))
    context = np.ascontiguousarray(np.asarray(context, dtype=np.float32))
    Wq = np.ascontiguousarray(np.asarray(Wq, dtype=np.float32))
    Wk = np.ascontiguousarray(np.asarray(Wk, dtype=np.float32))
    Wv = np.ascontiguousarray(np.asarray(Wv, dtype=np.float32))
    Wo = np.ascontiguousarray(np.asarray(Wo, dtype=np.float32))
    bo = np.ascontiguousarray(np.asarray(bo, dtype=np.float32))

    nc = _get_nc()
    in_maps = [
        {
            "x": x[i * BPC:(i + 1) * BPC],
            "context": context[i * BPC:(i + 1) * BPC],
            "Wq": Wq, "Wk": Wk, "Wv": Wv, "Wo": Wo, "bo": bo,
        }
        for i in range(CORES)
    ]
    res = run_bass_kernel_spmd(nc, in_maps, core_ids=list(range(CORES)),
                               trace=False)
    out = np.concatenate([res.results[i]["out"] for i in range(CORES)], axis=0)
    att = np.concatenate([res.results[i]["att"] for i in range(CORES)], axis=0)
    return out, att


# revision 23
# speedup vs baseline: 1.1927x; 1.1927x over previous
"""Trainium2 Bass kernel for batched dense attention.

Problem shapes (hardcoded): B=16, Lq=Lk=2048, E=1024, f32 I/O.
Sharding: batch dim across 8 NeuronCores (2 batches per core), no
communication. Each core computes, per batch:
    Q = ctx @ Wq^T ; K = x @ Wk^T ; V = x @ Wv^T
    att = softmax(Q K^T / 32) ; out = (att @ V) @ Wo^T + bo
Returns (out, att) like the reference.

Compute is bf16 on TensorE (fp32 PSUM accumulation). Input-side
transposes (weights/x/ctx, needed to put the contraction dim on
partitions) run on the TensorEngine as f32 PE-transposes, inline with
the matmul stream; P transposes ride the DMA X-bar from the Sync FIFO
(the only X-bar user in the attention phase). Softmax skips the
max-subtraction (scores are ~N(0,1); exp cannot overflow) and defers
the 1/rowsum normalisation of the out-path to the final projection
epilogue, where it is a per-partition scalar. Attention supers are
software-pipelined: super s's O/out-proj matmuls are emitted after
super s+1's S matmuls.
"""

import numpy as np

import concourse.bass as bass
import concourse.mybir as mybir
import concourse.tile as tile
from concourse import bacc
from concourse.bass_utils import run_bass_kernel_spmd
from concourse.masks import make_identity

F32 = mybir.dt.float32
BF16 = mybir.dt.bfloat16

B = 16
CORES = 8
BPC = B // CORES          # batches per core
LQ = 2048
LK = 2048
E = 1024
P = 128
NE = E // P               # 8  e-chunks
NF = E // P               # 8  f-chunks
NKC = LK // P             # 16 k-chunks
NSL = LK // 512           # 4  k-slices of 512
NSUP = LQ // 512          # 4  q-supers of 512
SCALE = 1.0 / 32.0        # 1/sqrt(E)


def build():
    nc = bacc.Bacc("TRN2", target_bir_lowering=False, debug=False)

    x = nc.dram_tensor("x", [BPC, LK, E], F32, kind="ExternalInput").ap()
    ctx = nc.dram_tensor("context", [BPC, LQ, E], F32, kind="ExternalInput").ap()
    Wq = nc.dram_tensor("Wq", [E, E], F32, kind="ExternalInput").ap()
    Wk = nc.dram_tensor("Wk", [E, E], F32, kind="ExternalInput").ap()
    Wv = nc.dram_tensor("Wv", [E, E], F32, kind="ExternalInput").ap()
    Wo = nc.dram_tensor("Wo", [E, E], F32, kind="ExternalInput").ap()
    bo = nc.dram_tensor("bo", [E], F32, kind="ExternalInput").ap()
    out = nc.dram_tensor("out", [BPC, LQ, E], F32, kind="ExternalOutput").ap()
    att = nc.dram_tensor("att", [BPC, LQ, LK], F32, kind="ExternalOutput").ap()

    # DRAM scratch holding transposed (e-major) bf16 weights for batch 1.
    wT_scr = nc.dram_tensor("wT_scr", [4, P, NE, E], BF16).ap()

    with tile.TileContext(nc) as tc:
        with tc.tile_pool(name="const", bufs=1) as const:
            bo_bc = const.tile([P, E], F32, tag="bo")
            bo_b = bass.AP(tensor=bo.tensor, offset=bo.offset,
                           ap=[[0, P]] + list(bo.ap))
            nc.gpsimd.dma_start(out=bo_bc, in_=bo_b)
            idf = const.tile([P, P], F32, tag="idf")
            make_identity(nc, idf)

            with tc.tile_pool(name="qkv", bufs=1) as qkv:
                for b in range(BPC):
                    QT = qkv.tile([P, NF, LQ], BF16, tag="QT")
                    KT = qkv.tile([P, NF, LK], BF16, tag="KT")
                    Vt = qkv.tile([P, NKC, E], BF16, tag="V")

                    # WoT outlives the projection phase (used in attention).
                    with tc.tile_pool(name=f"wo{b}", bufs=1) as wop:
                        WoT = wop.tile([P, NE, E], BF16, tag="WoT")
                        attention_batch(nc, tc, b, x, ctx, Wq, Wk, Wv, Wo,
                                        wT_scr, out, att, bo_bc, idf,
                                        QT, KT, Vt, WoT)

    nc.compile()
    return nc


def attention_batch(nc, tc, b, x, ctx, Wq, Wk, Wv, Wo, wT_scr,
                    out, att, bo_bc, idf, QT, KT, Vt, WoT):
    # ================= projection phase =================
    with tc.tile_pool(name=f"stg{b}", bufs=4) as stg, \
         tc.tile_pool(name=f"wt{b}", bufs=1) as wt, \
         tc.tile_pool(name=f"xt{b}", bufs=2) as xtp, \
         tc.tile_pool(name=f"pa{b}", bufs=6, space="PSUM") as pacc, \
         tc.tile_pool(name=f"pt{b}", bufs=2, space="PSUM") as ptr:

        WkT = wt.tile([P, NE, E], BF16, tag="WkT")
        WvT = wt.tile([P, NE, E], BF16, tag="WvT")
        WqT = wt.tile([P, NE, E], BF16, tag="WqT")

        copy_flip = [0]

        def pe_transpose(dst, src_f32, ec):
            """PE f32 transpose of one [P, P] block into bf16 dst, with the
            PSUM->SBUF copy alternating between DVE and ACT."""
            pt = ptr.tile([P, P], F32, tag="ptr")
            nc.tensor.transpose(pt, src_f32[:, ec * P:(ec + 1) * P], idf)
            eng = nc.vector if copy_flip[0] % 2 == 0 else nc.scalar
            copy_flip[0] += 1
            if eng is nc.vector:
                nc.vector.tensor_copy(dst, pt)
            else:
                nc.scalar.copy(dst, pt)

        def w_blocks(wi, wap, wT):
            """Stage f32 rows of a weight, PE-transpose into bf16 wT."""
            for fb in range(NF):
                wrow = stg.tile([P, E], F32, tag="xs")
                nc.scalar.dma_start(out=wrow, in_=wap[fb * P:(fb + 1) * P, :])
                for ec in range(NE):
                    pe_transpose(wT[:, ec, fb * P:(fb + 1) * P], wrow, ec)

        def w_load(wi, wT, fbs):
            """Load a transposed weight from DRAM scratch (HWDGE queues)."""
            for ec in range(NE):
                nc.scalar.dma_start(out=wT[:, ec, :], in_=wT_scr[wi, :, ec, :])

        def stage_T(src_dram, row0, xT, kb):
            """DRAM f32 rows -> bf16 -> X-bar transpose into xT[:, :, kb]."""
            xs = stg.tile([P, E], F32, tag="xs")
            nc.scalar.dma_start(out=xs, in_=src_dram[b, row0:row0 + P, :])
            xb = stg.tile([P, E], BF16, tag="xb")
            nc.vector.tensor_copy(xb, xs)
            nc.sync.dma_start_transpose(xT[:, :, kb * P:(kb + 1) * P], xb)

        def x_slice(sl):
            xT = xtp.tile([P, NE, 512], BF16, tag="xT")
            for kb in range(4):
                stage_T(x, sl * 512 + kb * P, xT, kb)
            for fb in range(NF):
                acc = pacc.tile([P, 512], F32, tag="acc")
                for ec in range(NE):
                    nc.tensor.matmul(
                        acc, WkT[:, ec, fb * P:(fb + 1) * P], xT[:, ec, :],
                        start=(ec == 0), stop=(ec == NE - 1))
                nc.scalar.copy(KT[:, fb, sl * 512:(sl + 1) * 512], acc)
            for kb in range(4):
                for fh in range(2):
                    acc = pacc.tile([P, 512], F32, tag="acc")
                    for ec in range(NE):
                        nc.tensor.matmul(
                            acc, xT[:, ec, kb * P:(kb + 1) * P],
                            WvT[:, ec, fh * 512:(fh + 1) * 512],
                            start=(ec == 0), stop=(ec == NE - 1))
                    nc.scalar.copy(
                        Vt[:, sl * 4 + kb, fh * 512:(fh + 1) * 512], acc)

        def ctx_slice(sl):
            cT = xtp.tile([P, NE, 512], BF16, tag="xT")
            for qb in range(4):
                stage_T(ctx, sl * 512 + qb * P, cT, qb)
            for fb in range(NF):
                acc = pacc.tile([P, 512], F32, tag="acc")
                for ec in range(NE):
                    nc.tensor.matmul(
                        acc, WqT[:, ec, fb * P:(fb + 1) * P], cT[:, ec, :],
                        start=(ec == 0), stop=(ec == NE - 1))
                nc.scalar.copy(QT[:, fb, sl * 512:(sl + 1) * 512], acc)

        if b == 0:
            w_blocks(1, Wk, WkT)
            w_blocks(2, Wv, WvT)
        else:
            w_load(1, WkT, range(NF))
            w_load(2, WvT, range(NF))
        x_slice(0)
        x_slice(1)
        if b == 0:
            w_blocks(0, Wq, WqT)
        else:
            w_load(0, WqT, range(NF))
        ctx_slice(0)
        x_slice(2)
        ctx_slice(1)
        x_slice(3)
        if b == 0:
            w_blocks(3, Wo, WoT)
        else:
            w_load(3, WoT, range(NF))
        ctx_slice(2)
        ctx_slice(3)
        if b == 0:
            # spill transposed weights for batch 1 (read at its proj start)
            for wi, wT in ((0, WqT), (1, WkT), (2, WvT), (3, WoT)):
                for ec in range(NE):
                    nc.gpsimd.dma_start(out=wT_scr[wi, :, ec, :],
                                        in_=wT[:, ec, :])

    # ================= attention phase =================
    with tc.tile_pool(name=f"pp{b}", bufs=2) as pp, \
         tc.tile_pool(name=f"at{b}", bufs=2) as atp, \
         tc.tile_pool(name=f"ot{b}", bufs=2) as otp, \
         tc.tile_pool(name=f"af{b}", bufs=6) as afp, \
         tc.tile_pool(name=f"sm{b}", bufs=12) as small, \
         tc.tile_pool(name=f"pS{b}", bufs=3, space="PSUM") as psS, \
         tc.tile_pool(name=f"pO{b}", bufs=3, space="PSUM") as psO, \
         tc.tile_pool(name=f"pF{b}", bufs=2, space="PSUM") as psF:

        def softmax_super(sup):
            """S matmuls + exp + att store + P transposes for one q-super.
            Returns (attT, recs) consumed by o_proj_super."""
            attT = atp.tile([P, NKC, 512], BF16, tag="attT")
            recs = []
            for qq in range(4):
                qb = sup * 4 + qq
                Pt = pp.tile([P, LK], BF16, tag="P")
                prt = small.tile([P, NSL], F32, tag="prt")
                for sl in range(NSL):
                    S = psS.tile([P, 512], F32, tag="S")
                    for fc in range(NF):
                        nc.tensor.matmul(
                            S, QT[:, fc, qb * P:(qb + 1) * P],
                            KT[:, fc, sl * 512:(sl + 1) * 512],
                            start=(fc == 0), stop=(fc == NF - 1))
                    nc.scalar.activation(
                        out=Pt[:, sl * 512:(sl + 1) * 512], in_=S,
                        func=mybir.ActivationFunctionType.Exp,
                        scale=SCALE, accum_out=prt[:, sl:sl + 1])
                    if sl % 2 == 1:
                        # transpose the finished [P, 1024] half; the Sync
                        # FIFO carries only transposes in this phase
                        half = sl // 2
                        nc.sync.dma_start_transpose(
                            attT[:, half * NE:(half + 1) * NE,
                                 qq * P:(qq + 1) * P],
                            Pt[:, half * 1024:(half + 1) * 1024])
                rsum = small.tile([P, 1], F32, tag="rsum")
                nc.vector.reduce_sum(rsum, prt, axis=mybir.AxisListType.X)
                rec = small.tile([P, 1], F32, tag="rec")
                nc.vector.reciprocal(rec, rsum)
                recs.append(rec)
                for sl in range(NSL):
                    af = afp.tile([P, 512], F32, tag="af")
                    nc.vector.tensor_scalar_mul(
                        af, Pt[:, sl * 512:(sl + 1) * 512], rec)
                    nc.gpsimd.dma_start(
                        out=att[b, qb * P:(qb + 1) * P,
                                sl * 512:(sl + 1) * 512],
                        in_=af)
            return attT, recs

        def o_proj_super(sup, attT, recs):
            """attV matmuls + output projection for one q-super."""
            OT = otp.tile([P, NE, 512], BF16, tag="OT")
            for eb in range(NE):
                acc = psO.tile([P, 512], F32, tag="O")
                for kc in range(NKC):
                    nc.tensor.matmul(
                        acc, Vt[:, kc, eb * P:(eb + 1) * P], attT[:, kc, :],
                        start=(kc == 0), stop=(kc == NKC - 1))
                nc.vector.tensor_copy(OT[:, eb, :], acc)
            for qq in range(4):
                qb = sup * 4 + qq
                for fh in range(2):
                    acc = psF.tile([P, 512], F32, tag="F")
                    for ec in range(NE):
                        nc.tensor.matmul(
                            acc, OT[:, ec, qq * P:(qq + 1) * P],
                            WoT[:, ec, fh * 512:(fh + 1) * 512],
                            start=(ec == 0), stop=(ec == NE - 1))
                    ost = afp.tile([P, 512], F32, tag="af")
                    nc.vector.tensor_scalar_mul(ost, acc, recs[qq])
                    nc.vector.tensor_add(
                        ost, ost, bo_bc[:, fh * 512:(fh + 1) * 512])
                    nc.gpsimd.dma_start(
                        out=out[b, qb * P:(qb + 1) * P,
                                fh * 512:(fh + 1) * 512],
                        in_=ost)

        pending = None
        for sup in range(NSUP):
            cur = softmax_super(sup)
            if pending is not None:
                o_proj_super(sup - 1, *pending)
            pending = cur
        o_proj_super(NSUP - 1, *pending)


_CACHE = {}


def _get_nc():
    if "nc" not in _CACHE:
        _CACHE["nc"] = build()
    return _CACHE["nc"]


def kernel(x, context, Wq, Wk, Wv, Wo, bo):
    x = np.ascontiguousarray(np.asarray(x, dtype=np.float32))
    context = np.ascontiguousarray(np.asarray(context, dtype=np.float32))
    Wq = np.ascontiguousarray(np.asarray(Wq, dtype=np.float32))
    Wk = np.ascontiguousarray(np.asarray(Wk, dtype=np.float32))
    Wv = np.ascontiguousarray(np.asarray(Wv, dtype=np.float32))
    Wo = np.ascontiguousarray(np.asarray(Wo, dtype=np.float32))
    bo = np.ascontiguousarray(np.asarray(bo, dtype=np.float32))

    nc = _get_nc()
    in_maps = [
        {
            "x": x[i * BPC:(i + 1) * BPC],
            "context": context[i * BPC:(i + 1) * BPC],
            "Wq": Wq, "Wk": Wk, "Wv": Wv, "Wo": Wo, "bo": bo,
        }
        for i in range(CORES)
    ]
    res = run_bass_kernel_spmd(nc, in_maps, core_ids=list(range(CORES)),
                               trace=False)
    out = np.concatenate([res.results[i]["out"] for i in range(CORES)], axis=0)
    att = np.concatenate([res.results[i]["att"] for i in range(CORES)], axis=0)
    return out, att
